# revision 1
# baseline (speedup 1.0000x reference)
"""Causal MHA attention-out kernel for TRN2, head-sharded across 8 NeuronCores.

Reference computation (fp32):
    scores = (q @ k^T) / sqrt(64), causal mask, softmax
    z      = pattern @ v
    out    = sum_h z_h @ W_O[h] + b_O          (residual passed through)

Sharding: 16 heads -> 8 cores x 2 adjacent heads. Each core computes a
partial out (its 2 heads' contribution, both batches); host sums partials.

Per-core layout (per batch b):
  kT/qT  [128, 2048]   d-major (head0 -> partitions 0-63, head1 -> 64-127),
                       loaded via bf16 xbar DMA transpose (dma_start_transpose).
  scoresT[k,q] blocks: matmul(lhsT=kT[64h:,kb], rhs=qT[64h:,qchunk]) -- the
                       two heads use disjoint PE row halves (row-group packed).
  softmax: exp on ACT straight out of PSUM (scale=1/8 folded in); causal
           triangle zeroed on GPSIMD via affine_select; denominator comes from
           a ones-column folded into V (row 64 of zT = sum_k pattern).
  z^T via matmul(lhsT=v_aug, rhs=patternT); normalize via DVE reciprocal +
           gpsimd partition_broadcast; project with both heads stacked (K=128).
  The two batches interleave at q-chunk granularity so one batch's
  normalize/project tail hides under the other's QK/exp/PV work. All matmul
  operands are bf16 (1 cyc/row on PE; f32r measured 4x slower on real HW),
  accumulation stays fp32 in PSUM. PSUM budget is exactly 8 banks: 2x2-bank
  double-buffered score tiles + 4x1-bank z-accumulator/projection slots --
  measured to be the critical-path resource, do not rebalance casually.
"""

import numpy as np

import concourse.bass as bass
import concourse.mybir as mybir
from concourse import bacc
import concourse.tile as tile
from concourse.bass_utils import run_bass_kernel_spmd
from concourse.masks import make_identity

B = 2
S = 2048
D_MODEL = 1024
N_HEADS = 16
D_HEAD = 64
N_CORES = 8
HPC = 2  # heads per core
CW = HPC * D_HEAD  # 128 columns of q/k/v per core
NKB = S // 128  # 16 k-blocks
NQC = S // 512  # 4 q-chunks
INV_SCALE = 1.0 / 8.0  # 1/sqrt(64)

F32 = mybir.dt.float32
F32R = mybir.dt.float32r
MMDT = mybir.dt.bfloat16  # matmul operand dtype: guaranteed 1 cyc/row on PE

_CACHE = {}


def _build_bass(reps=None):
    nc = bacc.Bacc("TRN2", target_bir_lowering=False)

    q_d = nc.dram_tensor("q", [B, S, CW], MMDT, kind="ExternalInput")
    k_d = nc.dram_tensor("k", [B, S, CW], MMDT, kind="ExternalInput")
    v_d = nc.dram_tensor("v", [B, S, CW], MMDT, kind="ExternalInput")
    wo_d = nc.dram_tensor("wo", [CW, D_MODEL], MMDT, kind="ExternalInput")
    out_d = nc.dram_tensor("out", [B, S, D_MODEL], MMDT, kind="ExternalOutput")

    with tile.TileContext(nc) as tc:
        with (
            tc.tile_pool(name="const", bufs=1) as const_pool,
            tc.tile_pool(name="big", bufs=2) as big_pool,
            tc.tile_pool(name="stage", bufs=4) as stage_pool,
            tc.tile_pool(name="pat", bufs=8) as pat_pool,
            tc.tile_pool(name="osb", bufs=4) as osb_pool,
            tc.tile_pool(name="psc", bufs=2, space="PSUM") as psc_pool,
            tc.tile_pool(name="pz", bufs=4, space="PSUM") as pz_pool,
            tc.tile_pool(name="dscr", bufs=4, space="DRAM") as dram_pool,
        ):
            ident_f = const_pool.tile([128, 128], F32)
            make_identity(nc, ident_f)
            ident = const_pool.tile([128, 128], MMDT)
            nc.vector.tensor_copy(ident, ident_f)
            ones16 = const_pool.tile([128, NKB], F32)
            nc.gpsimd.memset(ones16, 1.0)
            tri_f = const_pool.tile([128, 128], F32)
            nc.gpsimd.memset(tri_f, 1.0)
            nc.gpsimd.affine_select(
                out=tri_f,
                in_=tri_f,
                compare_op=mybir.AluOpType.is_ge,
                fill=0.0,
                base=0,
                pattern=[[1, 128]],
                channel_multiplier=-1,
            )
            tri = const_pool.tile([128, 128], MMDT)
            nc.vector.tensor_copy(tri, tri_f)
            wo_sb = const_pool.tile([CW, D_MODEL], MMDT)
            nc.sync.dma_start(wo_sb, wo_d[:, :])

            import contextlib

            loop_cm = (
                tc.For_i(
                    0,
                    reps,
                    1,
                    hint_engines=(
                        mybir.EngineType.PE,
                        mybir.EngineType.DVE,
                        mybir.EngineType.Activation,
                        mybir.EngineType.Pool,
                        mybir.EngineType.SP,
                    ),
                    staggered_reset=True,
                )
                if reps
                else contextlib.nullcontext()
            )
            with loop_cm:
                _emit_body(nc, tc, locals())
    nc.compile()
    return nc


def _emit_body(nc, tc, env):
    (q_d, k_d, v_d, wo_d, out_d) = (
        env["q_d"], env["k_d"], env["v_d"], env["wo_d"], env["out_d"]
    )
    (const_pool, big_pool, stage_pool, pat_pool, osb_pool, psc_pool, pz_pool,
     dram_pool) = (
        env["const_pool"], env["big_pool"], env["stage_pool"], env["pat_pool"],
        env["osb_pool"], env["psc_pool"], env["pz_pool"], env["dram_pool"]
    )
    ident, ones16, wo_sb = env["ident"], env["ones16"], env["wo_sb"]
    tri = env["tri"]
    if True:
        if True:
            kTs, qTs, vbigs = [], [], []
            for b in range(B):
                kT = big_pool.tile([128, S], MMDT, tag="kT", name=f"kT{b}")
                qT = big_pool.tile([128, S], MMDT, tag="qT", name=f"qT{b}")
                # v packed per k-block as [v_h0 | ones | v_h1 | ones] (130 cols)
                vbig = big_pool.tile([128, NKB * 130], MMDT, tag="vb", name=f"vb{b}")
                kTs.append(kT); qTs.append(qT); vbigs.append(vbig)
                # bf16 enables the xbar DMA transpose: one transposing DMA
                # per tensor replaces PE transposes + DVE copies entirely
                for src_, dstT in ((k_d, kT), (q_d, qT)):
                    for c4 in range(4):
                        nc.sync.dma_start_transpose(
                            dstT[:, c4 * 512 : (c4 + 1) * 512],
                            src_[b, c4 * 512 : (c4 + 1) * 512, :],
                        )
                v3 = vbig.rearrange("p (t c) -> p t c", c=130)
                nc.sync.dma_start(
                    v3[:, :, 0:64],
                    v_d[b].rearrange("(t p) c -> p t c", p=128)[:, :, 0:64],
                )
                nc.sync.dma_start(
                    v3[:, :, 65:129],
                    v_d[b].rearrange("(t p) c -> p t c", p=128)[:, :, 64:128],
                )
                nc.vector.tensor_copy(v3[:, :, 64], ones16)
                nc.vector.tensor_copy(v3[:, :, 129], ones16)

            # interleave the two batches so one batch's softmax/projection
            # tail overlaps the other batch's QK/exp/PV work
            for qc in range(NQC):
              for b in range(B):
                kT, qT, vbig = kTs[b], qTs[b], vbigs[b]
                if True:
                    # PSUM accumulators: zT_aug rows 0-63 = z, row 64 = denom
                    zacc = [
                        pz_pool.tile([65, 512], F32, tag="z", name=f"zacc{b}_{qc}_{h}")
                        for h in range(HPC)
                    ]
                    last_kb = 4 * qc + 3
                    pats = {}
                    for g in range(2 * qc + 2):
                        kbs = (2 * g, 2 * g + 1)
                        # QK^T: scoresT[k, q], heads row-group packed
                        sc_h = [
                            psc_pool.tile([128, 1024], F32, tag="sc", name=f"sc{hh}")
                            for hh in range(HPC)
                        ]
                        for half, kb in enumerate(kbs):
                            dd = kb - 4 * qc
                            s = 128 * dd if dd > 0 else 0
                            for h in range(HPC):
                                nc.tensor.matmul(
                                    sc_h[h][:, half * 512 + s : (half + 1) * 512],
                                    lhsT=kT[
                                        64 * h : 64 * h + 64, kb * 128 : (kb + 1) * 128
                                    ],
                                    rhs=qT[
                                        64 * h : 64 * h + 64,
                                        qc * 512 + s : (qc + 1) * 512,
                                    ],
                                    start=True,
                                    stop=True,
                                )
                        # exp (ACT reads PSUM, scale=1/8 folded in)
                        d0 = kbs[0] - 4 * qc
                        for h in range(HPC):
                            pt = pat_pool.tile([128, 1024], MMDT, tag="pat", name=f"pat{g}_{h}")
                            pats[(g, h)] = pt
                            if d0 < 0:
                                eranges = [(0, 1024)]
                            else:
                                eranges = [
                                    (d0 * 128, 512),
                                    (512 + (d0 + 1) * 128, 1024),
                                ]
                            for e0, e1 in eranges:
                                nc.scalar.activation(
                                    pt[:, e0:e1],
                                    sc_h[h][:, e0:e1],
                                    mybir.ActivationFunctionType.Exp,
                                    scale=INV_SCALE,
                                )
                            # causal 128x128 triangle zeroing on idle GPSIMD
                            for half, kb in enumerate(kbs):
                                dd = kb - 4 * qc
                                if dd < 0:
                                    continue
                                st = half * 512 + dd * 128
                                ap = pt[:, st : st + 128]
                                # keep where j - p >= 0 (i.e. p <= j)
                                nc.gpsimd.affine_select(
                                    out=ap,
                                    in_=ap,
                                    compare_op=mybir.AluOpType.is_ge,
                                    fill=0.0,
                                    base=0,
                                    pattern=[[1, 128]],
                                    channel_multiplier=-1,
                                )
                        # PV: zT_aug += v_aug^T @ patternT
                        for half, kb in enumerate(kbs):
                            dd = kb - 4 * qc
                            s = 128 * dd if dd > 0 else 0
                            for h in range(HPC):
                                nc.tensor.matmul(
                                    zacc[h][:, s:512],
                                    lhsT=vbig[
                                        :, kb * 130 + 65 * h : kb * 130 + 65 * h + 65
                                    ],
                                    rhs=pats[(g, h)][
                                        :, half * 512 + s : (half + 1) * 512
                                    ],
                                    start=(kb == 0),
                                    stop=(kb == last_kb),
                                )
                    # normalize: zT = zT / denom
                    zsb = stage_pool.tile([128, 512], MMDT, tag="zsb", name=f"zsb{b}_{qc}")
                    for h in range(HPC):
                        r_sb = stage_pool.tile([1, 512], F32, tag="r")
                        nc.vector.reciprocal(r_sb, zacc[h][64:65, :])
                        rb = stage_pool.tile([64, 512], F32, tag="rb")
                        nc.gpsimd.partition_broadcast(rb, r_sb)
                        nc.vector.tensor_mul(
                            zsb[64 * h : 64 * h + 64, :],
                            zacc[h][0:64, :],
                            rb,
                        )
                    # output projection, both heads stacked (K=128)
                    for qb in range(4):
                        osb = osb_pool.tile([128, D_MODEL], MMDT, tag="osb")
                        for mch in range(2):
                            op = pz_pool.tile([128, 512], F32, tag="z", name=f"op{b}_{qc}_{qb}_{mch}")
                            nc.tensor.matmul(
                                op,
                                lhsT=zsb[:, qb * 128 : (qb + 1) * 128],
                                rhs=wo_sb[:, mch * 512 : (mch + 1) * 512],
                                start=True,
                                stop=True,
                            )
                            nc.vector.tensor_copy(
                                osb[:, mch * 512 : (mch + 1) * 512], op
                            )
                        r0 = qc * 512 + qb * 128
                        nc.sync.dma_start(out_d[b, r0 : r0 + 128, :], osb)


def make_in_maps(q, k, v, W_O):
    import ml_dtypes

    bf16 = ml_dtypes.bfloat16
    q = np.asarray(q, dtype=np.float32).astype(bf16)
    k = np.asarray(k, dtype=np.float32).astype(bf16)
    v = np.asarray(v, dtype=np.float32).astype(bf16)
    W_O = np.asarray(W_O, dtype=np.float32).astype(bf16)
    in_maps = []
    for c in range(N_CORES):
        cols = slice(c * CW, (c + 1) * CW)
        in_maps.append(
            {
                "q": np.ascontiguousarray(q[:, :, cols]),
                "k": np.ascontiguousarray(k[:, :, cols]),
                "v": np.ascontiguousarray(v[:, :, cols]),
                "wo": np.ascontiguousarray(
                    W_O[c * HPC : (c + 1) * HPC].reshape(CW, D_MODEL)
                ),
            }
        )
    return in_maps


def get_nc():
    if "nc" not in _CACHE:
        _CACHE["nc"] = _build_bass()
    return _CACHE["nc"]


def kernel(q, k, v, residual, W_O, b_O):
    nc = get_nc()
    in_maps = make_in_maps(q, k, v, W_O)
    res = run_bass_kernel_spmd(nc, in_maps, core_ids=list(range(N_CORES)))
    out = res.results[0]["out"].astype(np.float64)
    for r in res.results[1:]:
        out += r["out"].astype(np.float64)
    out = (out + np.asarray(b_O, dtype=np.float64)[None, None, :]).astype(np.float32)
    return out, np.asarray(residual)

